# revision 8
# baseline (speedup 1.0000x reference)
"""Trainium2 Bass kernel for nn_ComposedFeatureTransformer (embedding lookup).

out_s[b, :] = bias + sum_k values_s[b, k] * merged_weight[indices_s[b, k], :]
for s in {0, 1}.

Strategy: data-parallel over the batch dim across 8 NeuronCores (512 rows
per core, both feature sets on every core; 8 tiles of 128 batch rows each).
The weight table is staged in HBM as fp16 with the bias appended as row V,
so a single indirect (gather) DMA per tile fetches all 32 feature rows plus
the bias row (33 x 128 = 4224 descriptors in one SWDGE instruction), and a
single strided DVE add-reduction (fp16 in, fp32 accumulate) collapses the
33 gathered rows into the output tile. feature_values are all ones for this
problem (spec fill=ones), so no scaling is needed; a general fallback path
applies the per-(batch,k) scale on ACT when any value differs from 1.
"""

import numpy as np

import concourse.bacc as bacc
import concourse.bass as bass
import concourse.mybir as mybir
import concourse.tile as tile
from concourse.bass_utils import run_bass_kernel_spmd

N_CORES = 8
BATCH = 4096
PER_CORE = BATCH // N_CORES  # 512 rows per feature set per core
K = 32
KB = K + 1  # 32 feature rows + 1 bias pseudo-row
V = 45056
D = 1032
P = 128
ROWS_PER_CORE = 2 * PER_CORE  # both feature sets: 1024
N_TILES = ROWS_PER_CORE // P  # 8
GATHER_CHUNK = 1  # idx columns per indirect DMA (HW ucode only supports one offset per partition)

TRACE = False  # set by test harness to collect an NTFF profile
LAST_RESULT = None  # BassKernelResults of the last run (for profiling)

_NC_FAST = None
_NC_GEN = None


def _build_fast(for_sim=False, bufs=2):
    """values==1 path: one fp16 gather (32 rows + bias row) and one DVE
    add-reduce per 128-batch tile."""
    kw = dict(target_bir_lowering=False, debug=True) if for_sim else {}
    nc = bacc.Bacc("TRN2", num_devices=N_CORES, **kw)
    f32 = mybir.dt.float32
    f16 = mybir.dt.float16
    i32 = mybir.dt.int32

    idx_d = nc.dram_tensor("idx", [ROWS_PER_CORE, KB], i32, kind="ExternalInput")
    w_d = nc.dram_tensor("weight", [V + 1, D], f16, kind="ExternalInput")
    out_d = nc.dram_tensor("out", [ROWS_PER_CORE, D], f32, kind="ExternalOutput")

    with tile.TileContext(nc) as tc:
        with (
            tc.tile_pool(name="io", bufs=bufs) as io_pool,
            tc.tile_pool(name="rows", bufs=bufs) as rows_pool,
            tc.tile_pool(name="acc", bufs=bufs) as acc_pool,
        ):
            for t in range(N_TILES):
                rs = slice(t * P, (t + 1) * P)
                idx_sb = io_pool.tile([P, KB], i32, tag="idx")
                nc.sync.dma_start(out=idx_sb[:], in_=idx_d[rs, :])
                rows = rows_pool.tile([P, KB, D], f16, tag="rows")
                # one offset per partition per instruction (HW ucode limit);
                # 2-D dest AP — a singleton middle axis also breaks the ucode
                for k in range(KB):
                    nc.gpsimd.indirect_dma_start(
                        out=rows[:, k, :],
                        out_offset=None,
                        in_=w_d[:],
                        in_offset=bass.IndirectOffsetOnAxis(
                            ap=idx_sb[:, k : k + 1], axis=0
                        ),
                    )
                # contiguous in-place pairwise tree: 33 chunks -> 1 (+bias chunk)
                for h in (16, 8, 4, 2):
                    nc.vector.tensor_add(
                        out=rows[:, 0:h, :],
                        in0=rows[:, 0:h, :],
                        in1=rows[:, h : 2 * h, :],
                    )
                acc = acc_pool.tile([P, D], f32, tag="acc")
                nc.vector.tensor_add(
                    out=rows[:, 1, :], in0=rows[:, 1, :], in1=rows[:, 32, :]
                )
                nc.vector.tensor_add(
                    out=acc[:], in0=rows[:, 0, :], in1=rows[:, 1, :]
                )
                nc.sync.dma_start(out=out_d[rs, :], in_=acc[:])

    nc.compile()
    return nc


def _build_general():
    """Fallback for arbitrary feature_values: f32 gather per k, ACT scale,
    DVE accumulate (bias folded into the k=0 accumulation)."""
    nc = bacc.Bacc("TRN2", debug=False, num_devices=N_CORES)
    f32 = mybir.dt.float32
    i32 = mybir.dt.int32
    n_tiles = PER_CORE // P

    idx_d = [
        nc.dram_tensor(f"idx{s}", [PER_CORE, K], i32, kind="ExternalInput")
        for s in range(2)
    ]
    val_d = [
        nc.dram_tensor(f"val{s}", [PER_CORE, K], f32, kind="ExternalInput")
        for s in range(2)
    ]
    w_d = nc.dram_tensor("weight", [V, D], f32, kind="ExternalInput")
    bias_d = nc.dram_tensor("bias_rep", [P, D], f32, kind="ExternalInput")
    out_d = [
        nc.dram_tensor(f"out{s}", [PER_CORE, D], f32, kind="ExternalOutput")
        for s in range(2)
    ]

    with tile.TileContext(nc) as tc:
        with (
            tc.tile_pool(name="const", bufs=1) as const_pool,
            tc.tile_pool(name="io", bufs=2) as io_pool,
            tc.tile_pool(name="rows", bufs=8) as rows_pool,
            tc.tile_pool(name="acc", bufs=2) as acc_pool,
        ):
            bias_sb = const_pool.tile([P, D], f32)
            nc.sync.dma_start(out=bias_sb[:], in_=bias_d[:])
            for s in range(2):
                for t in range(n_tiles):
                    rs = slice(t * P, (t + 1) * P)
                    idx_sb = io_pool.tile([P, K], i32, tag="idx")
                    val_sb = io_pool.tile([P, K], f32, tag="val")
                    nc.sync.dma_start(out=idx_sb[:], in_=idx_d[s][rs, :])
                    nc.sync.dma_start(out=val_sb[:], in_=val_d[s][rs, :])
                    acc = acc_pool.tile([P, D], f32, tag="acc")
                    for k in range(K):
                        rows = rows_pool.tile([P, D], f32, tag="rows")
                        nc.gpsimd.indirect_dma_start(
                            out=rows[:],
                            out_offset=None,
                            in_=w_d[:],
                            in_offset=bass.IndirectOffsetOnAxis(
                                ap=idx_sb[:, k : k + 1], axis=0
                            ),
                        )
                        scaled = rows_pool.tile([P, D], f32, tag="scaled")
                        nc.scalar.activation(
                            out=scaled[:],
                            in_=rows[:],
                            func=mybir.ActivationFunctionType.Copy,
                            scale=val_sb[:, k : k + 1],
                        )
                        if k == 0:
                            nc.vector.tensor_add(
                                out=acc[:], in0=scaled[:], in1=bias_sb[:]
                            )
                        else:
                            nc.vector.tensor_add(out=acc[:], in0=acc[:], in1=scaled[:])
                    nc.sync.dma_start(out=out_d[s][rs, :], in_=acc[:])

    nc.compile()
    return nc


def _get_fast():
    global _NC_FAST
    if _NC_FAST is None:
        _NC_FAST = _build_fast()
    return _NC_FAST


def _get_general():
    global _NC_GEN
    if _NC_GEN is None:
        _NC_GEN = _build_general()
    return _NC_GEN


def _pack_inputs_fast(idx0, idx1, w, b):
    """Per-core idx tiles with the bias pseudo-row column and fp16 table."""
    w16 = np.empty((V + 1, D), dtype=np.float16)
    w16[:V] = w
    w16[V] = b
    bias_col = np.full((PER_CORE, 1), V, dtype=np.int32)
    in_maps = []
    for c in range(N_CORES):
        rs = slice(c * PER_CORE, (c + 1) * PER_CORE)
        idx_all = np.concatenate(
            [
                np.concatenate([idx0[rs], bias_col], axis=1),
                np.concatenate([idx1[rs], bias_col], axis=1),
            ],
            axis=0,
        )
        in_maps.append(
            {
                "idx": np.ascontiguousarray(idx_all, dtype=np.int32),
                "weight": w16,
            }
        )
    return in_maps


def kernel(
    feature_indices_0,
    feature_values_0,
    feature_indices_1,
    feature_values_1,
    merged_weight,
    bias,
):
    global LAST_RESULT
    idx0 = np.ascontiguousarray(np.asarray(feature_indices_0, dtype=np.int32))
    idx1 = np.ascontiguousarray(np.asarray(feature_indices_1, dtype=np.int32))
    val0 = np.ascontiguousarray(np.asarray(feature_values_0, dtype=np.float32))
    val1 = np.ascontiguousarray(np.asarray(feature_values_1, dtype=np.float32))
    w = np.ascontiguousarray(np.asarray(merged_weight, dtype=np.float32))
    b = np.asarray(bias, dtype=np.float32)

    if np.all(val0 == 1.0) and np.all(val1 == 1.0):
        nc = _get_fast()
        in_maps = _pack_inputs_fast(idx0, idx1, w, b)
        res = run_bass_kernel_spmd(
            nc, in_maps, core_ids=list(range(N_CORES)), trace=TRACE
        )
        LAST_RESULT = res
        out0 = np.concatenate(
            [res.results[c]["out"][:PER_CORE] for c in range(N_CORES)], axis=0
        )
        out1 = np.concatenate(
            [res.results[c]["out"][PER_CORE:] for c in range(N_CORES)], axis=0
        )
        return out0, out1

    # general values: fall back to the f32 scale-and-accumulate path
    nc = _get_general()
    bias_rep = np.ascontiguousarray(np.broadcast_to(b[None, :], (P, D)))
    in_maps = []
    for c in range(N_CORES):
        rs = slice(c * PER_CORE, (c + 1) * PER_CORE)
        in_maps.append(
            {
                "idx0": idx0[rs],
                "val0": val0[rs],
                "idx1": idx1[rs],
                "val1": val1[rs],
                "weight": w,
                "bias_rep": bias_rep,
            }
        )
    res = run_bass_kernel_spmd(nc, in_maps, core_ids=list(range(N_CORES)), trace=TRACE)
    LAST_RESULT = res
    out0 = np.concatenate([res.results[c]["out0"] for c in range(N_CORES)], axis=0)
    out1 = np.concatenate([res.results[c]["out1"] for c in range(N_CORES)], axis=0)
    return out0, out1


# revision 11
# speedup vs baseline: 1.0178x; 1.0178x over previous
"""Trainium2 Bass kernel for nn_ComposedFeatureTransformer (embedding lookup).

out_s[b, :] = bias + sum_k values_s[b, k] * merged_weight[indices_s[b, k], :]
for s in {0, 1}.

Strategy: data-parallel over the batch dim across 8 NeuronCores (512 rows
per core, both feature sets on every core; 8 tiles of 128 batch rows each).
The weight table is staged in HBM as fp16 with the bias appended as row V,
so a single indirect (gather) DMA per tile fetches all 32 feature rows plus
the bias row (33 x 128 = 4224 descriptors in one SWDGE instruction), and a
single strided DVE add-reduction (fp16 in, fp32 accumulate) collapses the
33 gathered rows into the output tile. feature_values are all ones for this
problem (spec fill=ones), so no scaling is needed; a general fallback path
applies the per-(batch,k) scale on ACT when any value differs from 1.
"""

import numpy as np

import concourse.bacc as bacc
import concourse.bass as bass
import concourse.mybir as mybir
import concourse.tile as tile
from concourse.bass_utils import run_bass_kernel_spmd

N_CORES = 8
BATCH = 4096
PER_CORE = BATCH // N_CORES  # 512 rows per feature set per core
K = 32
KB = K + 1  # 32 feature rows + 1 bias pseudo-row
V = 45056
D = 1032
P = 128
ROWS_PER_CORE = 2 * PER_CORE  # both feature sets: 1024
N_TILES = ROWS_PER_CORE // P  # 8
DP = 1152  # padded row length for dma_gather (2304B, multiple of 256B)
GATHER_CHUNK = 1  # idx columns per indirect DMA (HW ucode only supports one offset per partition)

TRACE = False  # set by test harness to collect an NTFF profile
LAST_RESULT = None  # BassKernelResults of the last run (for profiling)

_NC_FAST = None
_NC_GEN = None


def _build_fast(for_sim=False, bufs=2):
    """values==1 path: 32 per-k fp16 indirect gathers per 128-batch tile
    (one offset per partition per instruction, 2-D dest APs -- both HW ucode
    limits), then a pairwise DVE tree whose first stage copies into a small
    t16 buffer so the big rows buffer frees early; bias from a const tile."""
    kw = dict(target_bir_lowering=False, debug=True) if for_sim else {}
    nc = bacc.Bacc("TRN2", num_devices=N_CORES, **kw)
    f32 = mybir.dt.float32
    f16 = mybir.dt.float16
    i32 = mybir.dt.int32

    idx_d = nc.dram_tensor("idx", [ROWS_PER_CORE, KB], i32, kind="ExternalInput")
    w_d = nc.dram_tensor("weight", [V + 1, D], f16, kind="ExternalInput")
    bias_d = nc.dram_tensor("bias_rep16", [P, D], f16, kind="ExternalInput")
    out_d = nc.dram_tensor("out", [ROWS_PER_CORE, D], f32, kind="ExternalOutput")

    with tile.TileContext(nc) as tc:
        with (
            tc.tile_pool(name="const", bufs=1) as const_pool,
            tc.tile_pool(name="io", bufs=3) as io_pool,
            tc.tile_pool(name="rows", bufs=2) as rows_pool,
            tc.tile_pool(name="t16", bufs=1) as t16_pool,
            tc.tile_pool(name="acc", bufs=2) as acc_pool,
        ):
            bias_sb = const_pool.tile([P, D], f16)
            nc.sync.dma_start(out=bias_sb[:], in_=bias_d[:])
            for t in range(N_TILES):
                rs = slice(t * P, (t + 1) * P)
                idx_sb = io_pool.tile([P, KB], i32, tag="idx")
                nc.sync.dma_start(out=idx_sb[:], in_=idx_d[rs, :])
                rows = rows_pool.tile([P, K, D], f16, tag="rows")
                for k in range(K):
                    nc.gpsimd.indirect_dma_start(
                        out=rows[:, k, :],
                        out_offset=None,
                        in_=w_d[:],
                        in_offset=bass.IndirectOffsetOnAxis(
                            ap=idx_sb[:, k : k + 1], axis=0
                        ),
                    )
                t16 = t16_pool.tile([P, 16, D], f16, tag="t16")
                nc.vector.tensor_add(
                    out=t16[:], in0=rows[:, 0:16, :], in1=rows[:, 16:32, :]
                )
                for h in (8, 4, 2):
                    nc.vector.tensor_add(
                        out=t16[:, 0:h, :],
                        in0=t16[:, 0:h, :],
                        in1=t16[:, h : 2 * h, :],
                    )
                nc.vector.tensor_add(
                    out=t16[:, 1, :], in0=t16[:, 1, :], in1=bias_sb[:]
                )
                acc = acc_pool.tile([P, D], f32, tag="acc")
                nc.vector.tensor_add(
                    out=acc[:], in0=t16[:, 0, :], in1=t16[:, 1, :]
                )
                nc.sync.dma_start(out=out_d[rs, :], in_=acc[:])

    nc.compile()
    return nc


def _build_general():
    """Fallback for arbitrary feature_values: f32 gather per k, ACT scale,
    DVE accumulate (bias folded into the k=0 accumulation)."""
    nc = bacc.Bacc("TRN2", debug=False, num_devices=N_CORES)
    f32 = mybir.dt.float32
    i32 = mybir.dt.int32
    n_tiles = PER_CORE // P

    idx_d = [
        nc.dram_tensor(f"idx{s}", [PER_CORE, K], i32, kind="ExternalInput")
        for s in range(2)
    ]
    val_d = [
        nc.dram_tensor(f"val{s}", [PER_CORE, K], f32, kind="ExternalInput")
        for s in range(2)
    ]
    w_d = nc.dram_tensor("weight", [V, D], f32, kind="ExternalInput")
    bias_d = nc.dram_tensor("bias_rep", [P, D], f32, kind="ExternalInput")
    out_d = [
        nc.dram_tensor(f"out{s}", [PER_CORE, D], f32, kind="ExternalOutput")
        for s in range(2)
    ]

    with tile.TileContext(nc) as tc:
        with (
            tc.tile_pool(name="const", bufs=1) as const_pool,
            tc.tile_pool(name="io", bufs=2) as io_pool,
            tc.tile_pool(name="rows", bufs=8) as rows_pool,
            tc.tile_pool(name="acc", bufs=2) as acc_pool,
        ):
            bias_sb = const_pool.tile([P, D], f32)
            nc.sync.dma_start(out=bias_sb[:], in_=bias_d[:])
            for s in range(2):
                for t in range(n_tiles):
                    rs = slice(t * P, (t + 1) * P)
                    idx_sb = io_pool.tile([P, K], i32, tag="idx")
                    val_sb = io_pool.tile([P, K], f32, tag="val")
                    nc.sync.dma_start(out=idx_sb[:], in_=idx_d[s][rs, :])
                    nc.sync.dma_start(out=val_sb[:], in_=val_d[s][rs, :])
                    acc = acc_pool.tile([P, D], f32, tag="acc")
                    for k in range(K):
                        rows = rows_pool.tile([P, D], f32, tag="rows")
                        nc.gpsimd.indirect_dma_start(
                            out=rows[:],
                            out_offset=None,
                            in_=w_d[:],
                            in_offset=bass.IndirectOffsetOnAxis(
                                ap=idx_sb[:, k : k + 1], axis=0
                            ),
                        )
                        scaled = rows_pool.tile([P, D], f32, tag="scaled")
                        nc.scalar.activation(
                            out=scaled[:],
                            in_=rows[:],
                            func=mybir.ActivationFunctionType.Copy,
                            scale=val_sb[:, k : k + 1],
                        )
                        if k == 0:
                            nc.vector.tensor_add(
                                out=acc[:], in0=scaled[:], in1=bias_sb[:]
                            )
                        else:
                            nc.vector.tensor_add(out=acc[:], in0=acc[:], in1=scaled[:])
                    nc.sync.dma_start(out=out_d[s][rs, :], in_=acc[:])

    nc.compile()
    return nc


_NC_HYB = {}


def _build_hybrid(k_s, for_sim=False):
    """values==1 path, hybrid gather: per-batch-sorted indices; first k_s
    columns (all < 32768 by construction) via one int16 dma_gather per tile,
    remaining columns + bias row via per-k indirect DMAs (2-D dest APs)."""
    kw = dict(target_bir_lowering=False, debug=True) if for_sim else {}
    nc = bacc.Bacc("TRN2", num_devices=N_CORES, **kw)
    f32 = mybir.dt.float32
    f16 = mybir.dt.float16
    i32 = mybir.dt.int32
    i16 = mybir.dt.int16

    idx32_d = nc.dram_tensor("idx32", [ROWS_PER_CORE, KB], i32, kind="ExternalInput")
    idx16_d = nc.dram_tensor("idx16", [N_TILES, P, k_s * 8], i16, kind="ExternalInput")
    w_d = nc.dram_tensor("weight", [V + 1, DP], f16, kind="ExternalInput")
    out_d = nc.dram_tensor("out", [ROWS_PER_CORE, D], f32, kind="ExternalOutput")

    with tile.TileContext(nc) as tc:
        with (
            tc.tile_pool(name="io", bufs=2) as io_pool,
            tc.tile_pool(name="rows", bufs=2) as rows_pool,
            tc.tile_pool(name="acc", bufs=2) as acc_pool,
        ):
            for t in range(N_TILES):
                rs = slice(t * P, (t + 1) * P)
                idx16_sb = io_pool.tile([P, k_s * 8], i16, tag="idx16")
                nc.sync.dma_start(out=idx16_sb[:], in_=idx16_d[t])
                idx32_sb = io_pool.tile([P, KB], i32, tag="idx32")
                nc.sync.dma_start(out=idx32_sb[:], in_=idx32_d[rs, :])
                rows = rows_pool.tile([P, KB, DP], f16, tag="rows")
                nc.gpsimd.dma_gather(
                    out_ap=rows[:, 0:k_s, :],
                    in_ap=w_d[:],
                    idxs_ap=idx16_sb[:],
                    num_idxs=k_s * P,
                    num_idxs_reg=k_s * P,
                    elem_size=DP,
                    queue_num=0,
                )
                for k in range(k_s, KB):
                    nc.gpsimd.indirect_dma_start(
                        out=rows[:, k, 0:D],
                        out_offset=None,
                        in_=w_d[:],
                        in_offset=bass.IndirectOffsetOnAxis(
                            ap=idx32_sb[:, k : k + 1], axis=0
                        ),
                    )
                for h in (16, 8, 4, 2):
                    nc.vector.tensor_add(
                        out=rows[:, 0:h, 0:D],
                        in0=rows[:, 0:h, 0:D],
                        in1=rows[:, h : 2 * h, 0:D],
                    )
                acc = acc_pool.tile([P, D], f32, tag="acc")
                nc.vector.tensor_add(
                    out=rows[:, 1, 0:D], in0=rows[:, 1, 0:D], in1=rows[:, 32, 0:D]
                )
                nc.vector.tensor_add(out=acc[:], in0=rows[:, 0, 0:D], in1=rows[:, 1, 0:D])
                nc.sync.dma_start(out=out_d[rs, :], in_=acc[:])

    nc.compile()
    return nc


def _choose_ks(idx0, idx1):
    srt = np.sort(np.concatenate([idx0, idx1], 0), axis=1)
    return int(min((srt < 32768).sum(axis=1).min(), K))


def _pack_inputs_hybrid(idx0, idx1, w, b, k_s):
    w16 = np.zeros((V + 1, DP), dtype=np.float16)
    w16[:V, :D] = w
    w16[V, :D] = b
    bias_col = np.full((PER_CORE, 1), V, dtype=np.int32)
    in_maps = []
    for c in range(N_CORES):
        rs = slice(c * PER_CORE, (c + 1) * PER_CORE)
        idx_all = np.concatenate(
            [
                np.concatenate([idx0[rs], bias_col], axis=1),
                np.concatenate([idx1[rs], bias_col], axis=1),
            ],
            axis=0,
        )
        srt = np.sort(idx_all, axis=1).astype(np.int32)  # bias lands last
        idx16 = np.empty((N_TILES, P, k_s * 8), dtype=np.int16)
        for t in range(N_TILES):
            # k-major list: position j = k*128 + b -> srt[tile_b, k]
            lst = srt[t * P : (t + 1) * P, :k_s].T.reshape(-1).astype(np.int16)
            wrapped = lst.reshape(-1, 16)  # slot s holds lst[s*16:(s+1)*16]
            idx16[t] = np.tile(wrapped.T, (8, 1))  # replicate over 8 stripes
        in_maps.append(
            {"idx32": np.ascontiguousarray(srt), "idx16": idx16, "weight": w16}
        )
    return in_maps


def _get_fast():
    global _NC_FAST
    if _NC_FAST is None:
        _NC_FAST = _build_fast()
    return _NC_FAST


def _get_general():
    global _NC_GEN
    if _NC_GEN is None:
        _NC_GEN = _build_general()
    return _NC_GEN


def _pack_inputs_fast(idx0, idx1, w, b):
    """Per-core idx tiles with the bias pseudo-row column and fp16 table."""
    w16 = np.empty((V + 1, D), dtype=np.float16)
    w16[:V] = w
    w16[V] = b
    bias_col = np.full((PER_CORE, 1), V, dtype=np.int32)
    in_maps = []
    for c in range(N_CORES):
        rs = slice(c * PER_CORE, (c + 1) * PER_CORE)
        idx_all = np.concatenate(
            [
                np.concatenate([idx0[rs], bias_col], axis=1),
                np.concatenate([idx1[rs], bias_col], axis=1),
            ],
            axis=0,
        )
        in_maps.append(
            {
                "idx": np.ascontiguousarray(idx_all, dtype=np.int32),
                "weight": w16,
                "bias_rep16": np.ascontiguousarray(
                    np.broadcast_to(b.astype(np.float16)[None, :], (P, D))
                ),
            }
        )
    return in_maps


def kernel(
    feature_indices_0,
    feature_values_0,
    feature_indices_1,
    feature_values_1,
    merged_weight,
    bias,
):
    global LAST_RESULT
    idx0 = np.ascontiguousarray(np.asarray(feature_indices_0, dtype=np.int32))
    idx1 = np.ascontiguousarray(np.asarray(feature_indices_1, dtype=np.int32))
    val0 = np.ascontiguousarray(np.asarray(feature_values_0, dtype=np.float32))
    val1 = np.ascontiguousarray(np.asarray(feature_values_1, dtype=np.float32))
    w = np.ascontiguousarray(np.asarray(merged_weight, dtype=np.float32))
    b = np.asarray(bias, dtype=np.float32)

    if np.all(val0 == 1.0) and np.all(val1 == 1.0):
        nc = _get_fast()
        in_maps = _pack_inputs_fast(idx0, idx1, w, b)
        res = run_bass_kernel_spmd(
            nc, in_maps, core_ids=list(range(N_CORES)), trace=TRACE
        )
        LAST_RESULT = res
        out0 = np.concatenate(
            [res.results[c]["out"][:PER_CORE] for c in range(N_CORES)], axis=0
        )
        out1 = np.concatenate(
            [res.results[c]["out"][PER_CORE:] for c in range(N_CORES)], axis=0
        )
        return out0, out1

    # general values: fall back to the f32 scale-and-accumulate path
    nc = _get_general()
    bias_rep = np.ascontiguousarray(np.broadcast_to(b[None, :], (P, D)))
    in_maps = []
    for c in range(N_CORES):
        rs = slice(c * PER_CORE, (c + 1) * PER_CORE)
        in_maps.append(
            {
                "idx0": idx0[rs],
                "val0": val0[rs],
                "idx1": idx1[rs],
                "val1": val1[rs],
                "weight": w,
                "bias_rep": bias_rep,
            }
        )
    res = run_bass_kernel_spmd(nc, in_maps, core_ids=list(range(N_CORES)), trace=TRACE)
    LAST_RESULT = res
    out0 = np.concatenate([res.results[c]["out0"] for c in range(N_CORES)], axis=0)
    out1 = np.concatenate([res.results[c]["out1"] for c in range(N_CORES)], axis=0)
    return out0, out1


# revision 12
# speedup vs baseline: 1.0354x; 1.0173x over previous
"""Trainium2 Bass kernel for nn_ComposedFeatureTransformer (embedding lookup).

out_s[b, :] = bias + sum_k values_s[b, k] * merged_weight[indices_s[b, k], :]
for s in {0, 1}.

Strategy: data-parallel over the batch dim across 8 NeuronCores (512 rows
per core, both feature sets on every core). The 186 MB weight table stays
in each core's HBM; rows are fetched with indirect (gather) DMA, 128 rows
per instruction (one offset per partition and 2-D dest APs -- the HW ucode
silently corrupts multi-column offset APs or singleton middle axes). ACT
applies the per-(batch,k) value scale, DVE accumulates, bias is folded into
the k=0 accumulation. The kernel is bound by the Pool engine's ~1.4 us
per-indirect-DMA SWDGE descriptor-generation cost (256 instructions/core),
not by HBM bandwidth.
"""

import numpy as np

import concourse.bacc as bacc
import concourse.bass as bass
import concourse.mybir as mybir
import concourse.tile as tile
from concourse.bass_utils import run_bass_kernel_spmd

N_CORES = 8
BATCH = 4096
PER_CORE = BATCH // N_CORES  # 512 rows per feature set per core
K = 32
KB = K + 1  # 32 feature rows + 1 bias pseudo-row
V = 45056
D = 1032
P = 128
ROWS_PER_CORE = 2 * PER_CORE  # both feature sets: 1024
N_TILES = ROWS_PER_CORE // P  # 8
DP = 1152  # padded row length for dma_gather (2304B, multiple of 256B)
GATHER_CHUNK = 1  # idx columns per indirect DMA (HW ucode only supports one offset per partition)

TRACE = False  # set by test harness to collect an NTFF profile
LAST_RESULT = None  # BassKernelResults of the last run (for profiling)

_NC_FAST = None
_NC_GEN = None


def _build_fast(for_sim=False, bufs=2):
    """values==1 path: 32 per-k fp16 indirect gathers per 128-batch tile
    (one offset per partition per instruction, 2-D dest APs -- both HW ucode
    limits), then a pairwise DVE tree whose first stage copies into a small
    t16 buffer so the big rows buffer frees early; bias from a const tile."""
    kw = dict(target_bir_lowering=False, debug=True) if for_sim else {}
    nc = bacc.Bacc("TRN2", num_devices=N_CORES, **kw)
    f32 = mybir.dt.float32
    f16 = mybir.dt.float16
    i32 = mybir.dt.int32

    idx_d = nc.dram_tensor("idx", [ROWS_PER_CORE, KB], i32, kind="ExternalInput")
    w_d = nc.dram_tensor("weight", [V + 1, D], f16, kind="ExternalInput")
    bias_d = nc.dram_tensor("bias_rep16", [P, D], f16, kind="ExternalInput")
    out_d = nc.dram_tensor("out", [ROWS_PER_CORE, D], f32, kind="ExternalOutput")

    with tile.TileContext(nc) as tc:
        with (
            tc.tile_pool(name="const", bufs=1) as const_pool,
            tc.tile_pool(name="io", bufs=3) as io_pool,
            tc.tile_pool(name="rows", bufs=2) as rows_pool,
            tc.tile_pool(name="t16", bufs=1) as t16_pool,
            tc.tile_pool(name="acc", bufs=2) as acc_pool,
        ):
            bias_sb = const_pool.tile([P, D], f16)
            nc.sync.dma_start(out=bias_sb[:], in_=bias_d[:])
            for t in range(N_TILES):
                rs = slice(t * P, (t + 1) * P)
                idx_sb = io_pool.tile([P, KB], i32, tag="idx")
                nc.sync.dma_start(out=idx_sb[:], in_=idx_d[rs, :])
                rows = rows_pool.tile([P, K, D], f16, tag="rows")
                for k in range(K):
                    nc.gpsimd.indirect_dma_start(
                        out=rows[:, k, :],
                        out_offset=None,
                        in_=w_d[:],
                        in_offset=bass.IndirectOffsetOnAxis(
                            ap=idx_sb[:, k : k + 1], axis=0
                        ),
                    )
                t16 = t16_pool.tile([P, 16, D], f16, tag="t16")
                nc.vector.tensor_add(
                    out=t16[:], in0=rows[:, 0:16, :], in1=rows[:, 16:32, :]
                )
                for h in (8, 4, 2):
                    nc.vector.tensor_add(
                        out=t16[:, 0:h, :],
                        in0=t16[:, 0:h, :],
                        in1=t16[:, h : 2 * h, :],
                    )
                nc.vector.tensor_add(
                    out=t16[:, 1, :], in0=t16[:, 1, :], in1=bias_sb[:]
                )
                acc = acc_pool.tile([P, D], f32, tag="acc")
                nc.vector.tensor_add(
                    out=acc[:], in0=t16[:, 0, :], in1=t16[:, 1, :]
                )
                nc.sync.dma_start(out=out_d[rs, :], in_=acc[:])

    nc.compile()
    return nc


def _build_general():
    """Fallback for arbitrary feature_values: f32 gather per k, ACT scale,
    DVE accumulate (bias folded into the k=0 accumulation)."""
    nc = bacc.Bacc("TRN2", debug=False, num_devices=N_CORES)
    f32 = mybir.dt.float32
    i32 = mybir.dt.int32
    n_tiles = PER_CORE // P

    idx_d = [
        nc.dram_tensor(f"idx{s}", [PER_CORE, K], i32, kind="ExternalInput")
        for s in range(2)
    ]
    val_d = [
        nc.dram_tensor(f"val{s}", [PER_CORE, K], f32, kind="ExternalInput")
        for s in range(2)
    ]
    w_d = nc.dram_tensor("weight", [V, D], f32, kind="ExternalInput")
    bias_d = nc.dram_tensor("bias_rep", [P, D], f32, kind="ExternalInput")
    out_d = [
        nc.dram_tensor(f"out{s}", [PER_CORE, D], f32, kind="ExternalOutput")
        for s in range(2)
    ]

    with tile.TileContext(nc) as tc:
        with (
            tc.tile_pool(name="const", bufs=1) as const_pool,
            tc.tile_pool(name="io", bufs=2) as io_pool,
            tc.tile_pool(name="rows", bufs=8) as rows_pool,
            tc.tile_pool(name="acc", bufs=2) as acc_pool,
        ):
            bias_sb = const_pool.tile([P, D], f32)
            nc.sync.dma_start(out=bias_sb[:], in_=bias_d[:])
            for s in range(2):
                for t in range(n_tiles):
                    rs = slice(t * P, (t + 1) * P)
                    idx_sb = io_pool.tile([P, K], i32, tag="idx")
                    val_sb = io_pool.tile([P, K], f32, tag="val")
                    nc.sync.dma_start(out=idx_sb[:], in_=idx_d[s][rs, :])
                    nc.sync.dma_start(out=val_sb[:], in_=val_d[s][rs, :])
                    acc = acc_pool.tile([P, D], f32, tag="acc")
                    for k in range(K):
                        rows = rows_pool.tile([P, D], f32, tag="rows")
                        nc.gpsimd.indirect_dma_start(
                            out=rows[:],
                            out_offset=None,
                            in_=w_d[:],
                            in_offset=bass.IndirectOffsetOnAxis(
                                ap=idx_sb[:, k : k + 1], axis=0
                            ),
                        )
                        scaled = rows_pool.tile([P, D], f32, tag="scaled")
                        nc.scalar.activation(
                            out=scaled[:],
                            in_=rows[:],
                            func=mybir.ActivationFunctionType.Copy,
                            scale=val_sb[:, k : k + 1],
                        )
                        if k == 0:
                            nc.vector.tensor_add(
                                out=acc[:], in0=scaled[:], in1=bias_sb[:]
                            )
                        else:
                            nc.vector.tensor_add(out=acc[:], in0=acc[:], in1=scaled[:])
                    nc.sync.dma_start(out=out_d[s][rs, :], in_=acc[:])

    nc.compile()
    return nc


_NC_HYB = {}


def _build_hybrid(k_s, for_sim=False):
    """values==1 path, hybrid gather: per-batch-sorted indices; first k_s
    columns (all < 32768 by construction) via one int16 dma_gather per tile,
    remaining columns + bias row via per-k indirect DMAs (2-D dest APs)."""
    kw = dict(target_bir_lowering=False, debug=True) if for_sim else {}
    nc = bacc.Bacc("TRN2", num_devices=N_CORES, **kw)
    f32 = mybir.dt.float32
    f16 = mybir.dt.float16
    i32 = mybir.dt.int32
    i16 = mybir.dt.int16

    idx32_d = nc.dram_tensor("idx32", [ROWS_PER_CORE, KB], i32, kind="ExternalInput")
    idx16_d = nc.dram_tensor("idx16", [N_TILES, P, k_s * 8], i16, kind="ExternalInput")
    w_d = nc.dram_tensor("weight", [V + 1, DP], f16, kind="ExternalInput")
    out_d = nc.dram_tensor("out", [ROWS_PER_CORE, D], f32, kind="ExternalOutput")

    with tile.TileContext(nc) as tc:
        with (
            tc.tile_pool(name="io", bufs=2) as io_pool,
            tc.tile_pool(name="rows", bufs=2) as rows_pool,
            tc.tile_pool(name="acc", bufs=2) as acc_pool,
        ):
            for t in range(N_TILES):
                rs = slice(t * P, (t + 1) * P)
                idx16_sb = io_pool.tile([P, k_s * 8], i16, tag="idx16")
                nc.sync.dma_start(out=idx16_sb[:], in_=idx16_d[t])
                idx32_sb = io_pool.tile([P, KB], i32, tag="idx32")
                nc.sync.dma_start(out=idx32_sb[:], in_=idx32_d[rs, :])
                rows = rows_pool.tile([P, KB, DP], f16, tag="rows")
                nc.gpsimd.dma_gather(
                    out_ap=rows[:, 0:k_s, :],
                    in_ap=w_d[:],
                    idxs_ap=idx16_sb[:],
                    num_idxs=k_s * P,
                    num_idxs_reg=k_s * P,
                    elem_size=DP,
                    queue_num=0,
                )
                for k in range(k_s, KB):
                    nc.gpsimd.indirect_dma_start(
                        out=rows[:, k, 0:D],
                        out_offset=None,
                        in_=w_d[:],
                        in_offset=bass.IndirectOffsetOnAxis(
                            ap=idx32_sb[:, k : k + 1], axis=0
                        ),
                    )
                for h in (16, 8, 4, 2):
                    nc.vector.tensor_add(
                        out=rows[:, 0:h, 0:D],
                        in0=rows[:, 0:h, 0:D],
                        in1=rows[:, h : 2 * h, 0:D],
                    )
                acc = acc_pool.tile([P, D], f32, tag="acc")
                nc.vector.tensor_add(
                    out=rows[:, 1, 0:D], in0=rows[:, 1, 0:D], in1=rows[:, 32, 0:D]
                )
                nc.vector.tensor_add(out=acc[:], in0=rows[:, 0, 0:D], in1=rows[:, 1, 0:D])
                nc.sync.dma_start(out=out_d[rs, :], in_=acc[:])

    nc.compile()
    return nc


def _choose_ks(idx0, idx1):
    srt = np.sort(np.concatenate([idx0, idx1], 0), axis=1)
    return int(min((srt < 32768).sum(axis=1).min(), K))


def _pack_inputs_hybrid(idx0, idx1, w, b, k_s):
    w16 = np.zeros((V + 1, DP), dtype=np.float16)
    w16[:V, :D] = w
    w16[V, :D] = b
    bias_col = np.full((PER_CORE, 1), V, dtype=np.int32)
    in_maps = []
    for c in range(N_CORES):
        rs = slice(c * PER_CORE, (c + 1) * PER_CORE)
        idx_all = np.concatenate(
            [
                np.concatenate([idx0[rs], bias_col], axis=1),
                np.concatenate([idx1[rs], bias_col], axis=1),
            ],
            axis=0,
        )
        srt = np.sort(idx_all, axis=1).astype(np.int32)  # bias lands last
        idx16 = np.empty((N_TILES, P, k_s * 8), dtype=np.int16)
        for t in range(N_TILES):
            # k-major list: position j = k*128 + b -> srt[tile_b, k]
            lst = srt[t * P : (t + 1) * P, :k_s].T.reshape(-1).astype(np.int16)
            wrapped = lst.reshape(-1, 16)  # slot s holds lst[s*16:(s+1)*16]
            idx16[t] = np.tile(wrapped.T, (8, 1))  # replicate over 8 stripes
        in_maps.append(
            {"idx32": np.ascontiguousarray(srt), "idx16": idx16, "weight": w16}
        )
    return in_maps


def _get_fast():
    global _NC_FAST
    if _NC_FAST is None:
        _NC_FAST = _build_fast()
    return _NC_FAST


def _get_general():
    global _NC_GEN
    if _NC_GEN is None:
        _NC_GEN = _build_general()
    return _NC_GEN


def _pack_inputs_fast(idx0, idx1, w, b):
    """Per-core idx tiles with the bias pseudo-row column and fp16 table."""
    w16 = np.empty((V + 1, D), dtype=np.float16)
    w16[:V] = w
    w16[V] = b
    bias_col = np.full((PER_CORE, 1), V, dtype=np.int32)
    in_maps = []
    for c in range(N_CORES):
        rs = slice(c * PER_CORE, (c + 1) * PER_CORE)
        idx_all = np.concatenate(
            [
                np.concatenate([idx0[rs], bias_col], axis=1),
                np.concatenate([idx1[rs], bias_col], axis=1),
            ],
            axis=0,
        )
        in_maps.append(
            {
                "idx": np.ascontiguousarray(idx_all, dtype=np.int32),
                "weight": w16,
                "bias_rep16": np.ascontiguousarray(
                    np.broadcast_to(b.astype(np.float16)[None, :], (P, D))
                ),
            }
        )
    return in_maps


def kernel(
    feature_indices_0,
    feature_values_0,
    feature_indices_1,
    feature_values_1,
    merged_weight,
    bias,
):
    global LAST_RESULT
    idx0 = np.ascontiguousarray(np.asarray(feature_indices_0, dtype=np.int32))
    idx1 = np.ascontiguousarray(np.asarray(feature_indices_1, dtype=np.int32))
    val0 = np.ascontiguousarray(np.asarray(feature_values_0, dtype=np.float32))
    val1 = np.ascontiguousarray(np.asarray(feature_values_1, dtype=np.float32))
    w = np.ascontiguousarray(np.asarray(merged_weight, dtype=np.float32))
    b = np.asarray(bias, dtype=np.float32)

    # The f32 scale-and-accumulate pipeline measures fastest on HW (the
    # per-instruction SWDGE cost of the 256 indirect gathers dominates every
    # design, so halving DMA bytes via fp16 does not pay; the baseline's
    # ACT/DVE chain overlaps the gather stream best). Use it for all inputs.
    nc = _get_general()
    bias_rep = np.ascontiguousarray(np.broadcast_to(b[None, :], (P, D)))
    in_maps = []
    for c in range(N_CORES):
        rs = slice(c * PER_CORE, (c + 1) * PER_CORE)
        in_maps.append(
            {
                "idx0": idx0[rs],
                "val0": val0[rs],
                "idx1": idx1[rs],
                "val1": val1[rs],
                "weight": w,
                "bias_rep": bias_rep,
            }
        )
    res = run_bass_kernel_spmd(nc, in_maps, core_ids=list(range(N_CORES)), trace=TRACE)
    LAST_RESULT = res
    out0 = np.concatenate([res.results[c]["out0"] for c in range(N_CORES)], axis=0)
    out1 = np.concatenate([res.results[c]["out1"] for c in range(N_CORES)], axis=0)
    return out0, out1
